# revision 1
# baseline (speedup 1.0000x reference)
"""OCSVM RBF-kernel scoring on Trainium2, data-parallel across 8 NeuronCores.

score[b] = sum_s c[s] * exp(-gamma * ||x_b - s_s||^2) - rho

Rewritten as:
    w[s]  = c[s] * exp(-gamma * s2[s])            (s2 = row norms of support vecs)
    E[b,s]= exp(2*gamma*cross[b,s] - gamma*x2[b])  (cross = X @ S^T)
    score = sum_s w[s] * E[b,s] - rho

Per-core (batch sharded 8 ways, B_loc=2048):
  - X^T / S^T are transposed host-side and loaded as float32r, so the tensor
    engine spends zero cycles on transposes and streams fp32 at 1 cyc/row
    (fp32r = hardware fast-fp32 mode, ~13-bit effective mantissa).
  - natural-layout fp32 copies are also uploaded for the row-norm reductions
    (VectorE accumulate), which need rows on partitions.
  - exp on ScalarE reads PSUM [128,2048] (4 banks) per instruction with
    per-partition scale=2*gamma, bias=-gamma*x2[b].
  - weighted reduction over s on VectorE (scalar_tensor_tensor accum_out),
    with w replicated across partitions via a DRAM bounce.
"""

import numpy as np

B_TOT = 16384
B_LOC = 2048
S_TOT = 8192
F = 512
P = 128
N_CORES = 8

FC = F // P            # 4 contraction chunks
NB = B_LOC // P        # 16 batch tiles per core
SUPER = 2048           # s-columns per super-tile resident in SBUF
N_SUP = S_TOT // SUPER  # 4
NBLK = SUPER // P      # 16 s row-blocks per super (norm path)
NT = 512               # matmul moving free dim (one PSUM bank)
EW = SUPER             # elementwise tile width (whole super, 4 PSUM banks)

MM_DT = "f32r"   # matmul operand dtype: f32r | f16 | bf16

_CACHE = {}


def _build(repeat=1, mm_dt=None):
    """Trace + compile the SPMD Bass program (cached).

    repeat > 1 wraps the compute in a hardware For_i loop that redoes
    identical work; used only for wall-clock amplification when benchmarking.
    """
    mm_dt = mm_dt or MM_DT
    key = (repeat, mm_dt)
    if key in _CACHE:
        return _CACHE[key]

    from contextlib import ExitStack

    import concourse.mybir as mybir
    import concourse.tile as tile
    from concourse import bacc
    from concourse.masks import make_identity

    f32 = mybir.dt.float32
    bf16 = mybir.dt.bfloat16
    MDT = {"f32r": mybir.dt.float32r, "f16": mybir.dt.float16,
           "bf16": mybir.dt.bfloat16}[mm_dt]
    FT = mybir.ActivationFunctionType
    OP = mybir.AluOpType

    nc = bacc.Bacc("TRN2", target_bir_lowering=False, debug=False)

    xt_d = nc.dram_tensor("xt", [F, B_LOC], MDT, kind="ExternalInput").ap()
    st_d = nc.dram_tensor("st", [F, S_TOT], MDT, kind="ExternalInput").ap()
    NDT = f32 if mm_dt == "f32r" else mybir.dt.float16
    xn_d = nc.dram_tensor("xn", [B_LOC, F], NDT, kind="ExternalInput").ap()
    sn_d = nc.dram_tensor("sn", [S_TOT, F], NDT, kind="ExternalInput").ap()
    c_d = nc.dram_tensor("c", [1, S_TOT], f32, kind="ExternalInput").ap()
    rho_d = nc.dram_tensor("rho", [1, 1], f32, kind="ExternalInput").ap()
    gam_d = nc.dram_tensor("gamma", [1, 1], f32, kind="ExternalInput").ap()
    out_d = nc.dram_tensor("out", [NB, P], f32, kind="ExternalOutput").ap()
    # bounce buffer to replicate the s-weights row across all 128 partitions
    wrow_d = nc.dram_tensor("w_bounce", [1, S_TOT], bf16).ap()

    # [F, N] viewed as [128, FC, N] (partition-major within each f-chunk)
    xt_v = xt_d.rearrange("(c p) b -> p c b", p=P)
    st_v = st_d.rearrange("(c p) s -> p c s", p=P)

    with tile.TileContext(nc) as tc, ExitStack() as ctx:
        const_p = ctx.enter_context(tc.tile_pool(name="const", bufs=1))
        fin_p = ctx.enter_context(tc.tile_pool(name="fin", bufs=1))
        xn_p = ctx.enter_context(tc.tile_pool(name="xn", bufs=3))
        sn_p = ctx.enter_context(tc.tile_pool(name="sn", bufs=4))
        sq_p = ctx.enter_context(tc.tile_pool(name="sq", bufs=2))
        xt_p = ctx.enter_context(tc.tile_pool(name="xt", bufs=1))
        st_p = ctx.enter_context(tc.tile_pool(name="st", bufs=2))
        w_p = ctx.enter_context(tc.tile_pool(name="w", bufs=1))
        row_p = ctx.enter_context(tc.tile_pool(name="row", bufs=1))
        e_p = ctx.enter_context(tc.tile_pool(name="e", bufs=3))
        scr_p = ctx.enter_context(tc.tile_pool(name="scr", bufs=2))
        ps = ctx.enter_context(tc.tile_pool(name="ps", bufs=2, space="PSUM"))

        # ---- constants (outside any repeat loop) ----
        ident = const_p.tile([P, P], f32)
        make_identity(nc, ident[:])
        gb = const_p.tile([P, 1], f32)
        nc.sync.dma_start(out=gb[:], in_=gam_d.partition_broadcast(P))
        rb = const_p.tile([P, 1], f32)
        nc.sync.dma_start(out=rb[:], in_=rho_d.partition_broadcast(P))
        two_g = const_p.tile([P, 1], f32)
        nc.scalar.mul(two_g[:], gb[:], 2.0)
        ng = const_p.tile([P, 1], f32)
        nc.scalar.mul(ng[:], gb[:], -1.0)

        x2_pt = fin_p.tile([P, NB], f32)
        bias_pt = fin_p.tile([P, NB], f32)
        parts = fin_p.tile([P, NB * N_SUP], f32)
        score = fin_p.tile([P, NB], f32)

        xt = xt_p.tile([P, FC, B_LOC], MDT)   # X^T, all 4 f-chunks
        w_bc = w_p.tile([P, S_TOT], bf16)     # w replicated across partitions

        def emit_main():
            # ---- X stage ----
            nc.sync.dma_start(out=xt[:], in_=xt_v)
            for t in range(NB):
                xn = xn_p.tile([P, F], NDT, tag="xn", name="xn")
                nc.scalar.dma_start(out=xn[:], in_=xn_d[t * P:(t + 1) * P, :])
                xsq = sq_p.tile([P, F], NDT, tag="sq", name="xsq")
                nc.vector.scalar_tensor_tensor(
                    out=xsq[:], in0=xn[:], scalar=1.0, in1=xn[:],
                    op0=OP.mult, op1=OP.mult, accum_out=x2_pt[:, t:t + 1])
            nc.vector.tensor_scalar_mul(bias_pt[:], x2_pt[:], ng[:])

            # ---- main loop over s super-tiles ----
            for u in range(N_SUP):
                st = st_p.tile([P, FC, SUPER], MDT, tag="st", name="st")
                nc.sync.dma_start(out=st[:], in_=st_v[:, :, u * SUPER:(u + 1) * SUPER])

                # row norms of this super's support vectors -> s2_pt
                s2_pt = row_p.tile([P, NBLK], f32, tag="s2pt", name="s2_pt")
                for j in range(NBLK):
                    q = u * NBLK + j
                    sn = sn_p.tile([P, F], NDT, tag="sn", name="sn")
                    nc.scalar.dma_start(out=sn[:], in_=sn_d[q * P:(q + 1) * P, :])
                    ssq = sq_p.tile([P, F], NDT, tag="sq", name="ssq")
                    nc.vector.scalar_tensor_tensor(
                        out=ssq[:], in0=sn[:], scalar=1.0, in1=sn[:],
                        op0=OP.mult, op1=OP.mult, accum_out=s2_pt[:, j:j + 1])

                # w chain: w[s] = c[s]*exp(-gamma*s2[s]) on one row, then
                # replicate to 128 partitions via DRAM bounce.
                s2t_ps = ps.tile([NBLK, P], f32, tag="pm", name="s2t_ps")
                nc.tensor.transpose(s2t_ps[:], s2_pt[:], ident[:])
                s2_rows = row_p.tile([NBLK, P], f32, tag="s2rows", name="s2_rows")
                nc.vector.tensor_copy(out=s2_rows[:], in_=s2t_ps[:])
                s2_row = row_p.tile([1, SUPER], f32, tag="s2row", name="s2_row")
                nc.sync.dma_start(out=s2_row[:], in_=s2_rows[:])
                c_sl = row_p.tile([1, SUPER], f32, tag="csl", name="c_sl")
                nc.sync.dma_start(out=c_sl[:], in_=c_d[:, u * SUPER:(u + 1) * SUPER])
                w_exp = row_p.tile([1, SUPER], f32, tag="wexp", name="w_exp")
                nc.scalar.activation(out=w_exp[:], in_=s2_row[:], func=FT.Exp,
                                     scale=ng[:1, :])
                w_sl = row_p.tile([1, SUPER], bf16, tag="wsl", name="w_sl")
                nc.vector.tensor_mul(w_sl[:], w_exp[:], c_sl[:])
                nc.sync.dma_start(out=wrow_d[:, u * SUPER:(u + 1) * SUPER],
                                  in_=w_sl[:])
                nc.sync.dma_start(
                    out=w_bc[:, u * SUPER:(u + 1) * SUPER],
                    in_=wrow_d[:, u * SUPER:(u + 1) * SUPER].partition_broadcast(P))

                # matmuls + exp + weighted reduce, one [128, 2048] group per t
                for t in range(NB):
                    pm = ps.tile([P, EW], f32, tag="pm", name="pm")
                    for fc in range(FC):
                        for h in range(EW // NT):
                            nc.tensor.matmul(
                                pm[:, h * NT:(h + 1) * NT],
                                xt[:, fc, t * P:(t + 1) * P],
                                st[:, fc, h * NT:(h + 1) * NT],
                                start=(fc == 0), stop=(fc == FC - 1))
                    et = e_p.tile([P, EW], bf16, tag="et", name="et")
                    nc.scalar.activation(out=et[:], in_=pm[:], func=FT.Exp,
                                         scale=two_g[:], bias=bias_pt[:, t:t + 1])
                    dead = scr_p.tile([P, EW], bf16, tag="dead", name="dead")
                    col = t * N_SUP + u
                    nc.vector.scalar_tensor_tensor(
                        out=dead[:], in0=et[:], scalar=1.0,
                        in1=w_bc[:, u * SUPER:(u + 1) * SUPER],
                        op0=OP.mult, op1=OP.mult,
                        accum_out=parts[:, col:col + 1])

            # ---- finale: reduce partials, subtract rho, transpose out ----
            pv = parts[:].rearrange("p (t k) -> p t k", k=N_SUP)
            nc.vector.tensor_reduce(out=score[:], in_=pv,
                                    axis=mybir.AxisListType.X, op=OP.add)
            nc.vector.tensor_scalar_sub(score[:], score[:], rb[:])
            sc_ps = ps.tile([NB, P], f32, tag="pm", name="sc_ps")
            nc.tensor.transpose(sc_ps[:], score[:], ident[:])
            sc_t = fin_p.tile([NB, P], f32, name="sc_t")
            nc.vector.tensor_copy(out=sc_t[:], in_=sc_ps[:])
            nc.sync.dma_start(out=out_d, in_=sc_t[:])

        if repeat == 1:
            emit_main()
        else:
            with tc.For_i(0, repeat, 1):
                emit_main()

    nc.compile()
    _CACHE[key] = nc
    return nc


def _in_maps(inputs, support_vectors, coefficients, rho, gamma, mm_dt=None):
    mm_dt = mm_dt or MM_DT
    if mm_dt == "f16":
        tdt = np.float16
    elif mm_dt == "bf16":
        import ml_dtypes
        tdt = ml_dtypes.bfloat16
    else:
        tdt = np.float32
    ndt = np.float32 if mm_dt == "f32r" else np.float16
    x = np.asarray(inputs, dtype=np.float32)
    s = np.asarray(support_vectors, dtype=np.float32)
    s_t = np.ascontiguousarray(s.T.astype(tdt))
    s_n = np.ascontiguousarray(s.astype(ndt))
    c = np.ascontiguousarray(np.asarray(coefficients, np.float32)).reshape(1, S_TOT)
    r = np.asarray(rho, dtype=np.float32).reshape(1, 1)
    g = np.asarray(gamma, dtype=np.float32).reshape(1, 1)
    maps = []
    for cid in range(N_CORES):
        xs = x[cid * B_LOC:(cid + 1) * B_LOC]
        maps.append({
            "xt": np.ascontiguousarray(xs.T.astype(tdt)),
            "xn": np.ascontiguousarray(xs.astype(ndt)),
            "st": s_t,
            "sn": s_n,
            "c": c,
            "rho": r,
            "gamma": g,
        })
    return maps


def kernel(inputs, support_vectors, coefficients, rho, gamma, _trace=False):
    from concourse.bass_utils import run_bass_kernel_spmd

    nc = _build()
    maps = _in_maps(inputs, support_vectors, coefficients, rho, gamma)
    res = run_bass_kernel_spmd(nc, maps, core_ids=list(range(N_CORES)),
                               trace=_trace)
    out = np.concatenate([np.asarray(r["out"], dtype=np.float32).reshape(B_LOC)
                          for r in res.results])
    if _trace:
        kernel.last_results = res
    return out



# revision 2
# speedup vs baseline: 13.5857x; 13.5857x over previous
"""OCSVM RBF-kernel scoring on Trainium2, 8 NeuronCores.

score[b] = sum_s c[s] * exp(-gamma * ||x_b - s_s||^2) - rho

Rewritten as:
    w[s]  = c[s] * exp(-gamma * s2[s])            (s2 = row norms of support vecs)
    E[b,s]= exp(2*gamma*cross[b,s] - gamma*x2[b])  (cross = X @ S^T)
    score = sum_s w[s] * E[b,s] - rho

The wall-clock cost of a kernel() call here is dominated by host->device
transfer over the axon tunnel (~60 MB/s), not device compute (~0.3 ms).
So the design minimizes wire bytes: every input byte crosses the wire
exactly once, in float16:

  - X is batch-sharded: each core receives its own [2048, 512] f16 slice
    in natural layout (16 MB total across cores).
  - S is *sharded* too: each core receives a distinct [1024, 512] f16
    slice (8 MB total) and the full [8192, 512] S is reassembled on-device
    with an AllGather over NeuronLink (DRAM->DRAM collective).
  - Norms (x2, s2) are computed on device from the same f16 data.
  - Transposed matmul operand layouts ([F, B] / [F, S], f on partitions)
    are produced by hardware DMA-transpose (2-byte dtype XBAR path) during
    the DRAM->SBUF load, so neither host nor PE spends time transposing.

Device math (per core, B_loc=2048):
  - cross = X @ S^T as 1024 f16 matmuls [128f,128b]x[128f,512s] -> PSUM.
  - exp on ScalarE reads PSUM [128, 2048] with per-partition scale=2*gamma,
    bias=-gamma*x2[b].
  - weighted reduction over s on VectorE (scalar_tensor_tensor accum_out),
    with w = c*exp(-gamma*s2) replicated across partitions via DRAM bounce.
"""

import numpy as np

B_TOT = 16384
B_LOC = 2048
S_TOT = 8192
S_SH = 1024            # per-core S shard (AllGather reassembles full S)
F = 512
P = 128
N_CORES = 8

FC = F // P            # 4 contraction chunks
NB = B_LOC // P        # 16 batch tiles per core
SUPER = 2048           # s-columns per processing group
N_SUP = S_TOT // SUPER  # 4
NBLK = SUPER // P      # 16 s row-blocks per super (norm path)
NT = 512               # matmul moving free dim (one PSUM bank)
EW = SUPER             # elementwise tile width (4 PSUM banks)
TR = 512               # rows per DMA-transpose load

_CACHE = {}


def _build():
    """Trace + compile the SPMD Bass program (cached)."""
    if "nc" in _CACHE:
        return _CACHE["nc"]

    from contextlib import ExitStack

    import concourse.mybir as mybir
    import concourse.tile as tile
    from concourse import bacc
    from concourse.masks import make_identity

    f32 = mybir.dt.float32
    f16 = mybir.dt.float16
    bf16 = mybir.dt.bfloat16
    FT = mybir.ActivationFunctionType
    OP = mybir.AluOpType

    nc = bacc.Bacc("TRN2", target_bir_lowering=False, debug=False)

    x_d = nc.dram_tensor("x", [B_LOC, F], f16, kind="ExternalInput").ap()
    s_d = nc.dram_tensor("s", [S_SH, F], f16, kind="ExternalInput").ap()
    c_d = nc.dram_tensor("c", [1, S_TOT], f32, kind="ExternalInput").ap()
    rho_d = nc.dram_tensor("rho", [1, 1], f32, kind="ExternalInput").ap()
    gam_d = nc.dram_tensor("gamma", [1, 1], f32, kind="ExternalInput").ap()
    out_d = nc.dram_tensor("out", [NB, P], f32, kind="ExternalOutput").ap()
    # bounce buffer to replicate the s-weights row across all 128 partitions
    wrow_d = nc.dram_tensor("w_bounce", [1, S_TOT], bf16).ap()

    with tile.TileContext(nc) as tc, ExitStack() as ctx:
        dram_p = ctx.enter_context(tc.tile_pool(name="dram", bufs=1, space="DRAM"))
        const_p = ctx.enter_context(tc.tile_pool(name="const", bufs=1))
        fin_p = ctx.enter_context(tc.tile_pool(name="fin", bufs=1))
        nat_p = ctx.enter_context(tc.tile_pool(name="nat", bufs=4))
        sq_p = ctx.enter_context(tc.tile_pool(name="sq", bufs=2))
        xt_p = ctx.enter_context(tc.tile_pool(name="xt", bufs=1))
        st_p = ctx.enter_context(tc.tile_pool(name="st", bufs=1))
        w_p = ctx.enter_context(tc.tile_pool(name="w", bufs=1))
        row_p = ctx.enter_context(tc.tile_pool(name="row", bufs=1))
        e_p = ctx.enter_context(tc.tile_pool(name="e", bufs=3))
        scr_p = ctx.enter_context(tc.tile_pool(name="scr", bufs=2))
        ps = ctx.enter_context(tc.tile_pool(name="ps", bufs=2, space="PSUM"))

        # ---- AllGather S shards into the full support-vector matrix ----
        sag_in = dram_p.tile([S_SH, F], f16)
        sag_out = dram_p.tile([S_TOT, F], f16)
        nc.gpsimd.dma_start(sag_in[:], s_d)
        nc.gpsimd.collective_compute(
            "AllGather",
            mybir.AluOpType.bypass,
            replica_groups=[list(range(N_CORES))],
            ins=[sag_in.opt()],
            outs=[sag_out.opt()],
        )

        # ---- constants ----
        ident = const_p.tile([P, P], f32)
        make_identity(nc, ident[:])
        gb = const_p.tile([P, 1], f32)
        nc.sync.dma_start(out=gb[:], in_=gam_d.partition_broadcast(P))
        rb = const_p.tile([P, 1], f32)
        nc.sync.dma_start(out=rb[:], in_=rho_d.partition_broadcast(P))
        two_g = const_p.tile([P, 1], f32)
        nc.scalar.mul(two_g[:], gb[:], 2.0)
        ng = const_p.tile([P, 1], f32)
        nc.scalar.mul(ng[:], gb[:], -1.0)

        x2_pt = fin_p.tile([P, NB], f32)
        bias_pt = fin_p.tile([P, NB], f32)
        parts = fin_p.tile([P, NB * N_SUP], f32)
        score = fin_p.tile([P, NB], f32)

        xt = xt_p.tile([P, FC, B_LOC], f16)   # X^T (f on partitions)
        st = st_p.tile([P, FC, S_TOT], f16)   # S^T (f on partitions)
        w_bc = w_p.tile([P, S_TOT], bf16)     # w replicated across partitions

        # ---- X stage: row norms + DMA-transposed matmul operand ----
        for t in range(NB):
            xn = nat_p.tile([P, F], f16, tag="xn", name="xn")
            nc.scalar.dma_start(out=xn[:], in_=x_d[t * P:(t + 1) * P, :])
            xsq = sq_p.tile([P, F], f16, tag="sq", name="xsq")
            nc.vector.scalar_tensor_tensor(
                out=xsq[:], in0=xn[:], scalar=1.0, in1=xn[:],
                op0=OP.mult, op1=OP.mult, accum_out=x2_pt[:, t:t + 1])
        for r in range(B_LOC // TR):
            for fc in range(FC):
                nc.sync.dma_start(
                    out=xt[:, fc, r * TR:(r + 1) * TR],
                    in_=x_d[r * TR:(r + 1) * TR, fc * P:(fc + 1) * P],
                    transpose=True)
        nc.vector.tensor_scalar_mul(bias_pt[:], x2_pt[:], ng[:])

        # ---- S stage (after AllGather): norms + w chain + S^T loads ----
        for u in range(N_SUP):
            # row norms of this super's support vectors -> s2_pt
            s2_pt = row_p.tile([P, NBLK], f32, tag="s2pt", name="s2_pt")
            for j in range(NBLK):
                q = u * NBLK + j
                sn = nat_p.tile([P, F], f16, tag="sn", name="sn")
                nc.scalar.dma_start(out=sn[:], in_=sag_out[q * P:(q + 1) * P, :])
                ssq = sq_p.tile([P, F], f16, tag="sq", name="ssq")
                nc.vector.scalar_tensor_tensor(
                    out=ssq[:], in0=sn[:], scalar=1.0, in1=sn[:],
                    op0=OP.mult, op1=OP.mult, accum_out=s2_pt[:, j:j + 1])
            for r in range(SUPER // TR):
                for fc in range(FC):
                    base = u * SUPER + r * TR
                    nc.sync.dma_start(
                        out=st[:, fc, base:base + TR],
                        in_=sag_out[base:base + TR, fc * P:(fc + 1) * P],
                        transpose=True)

            # w chain: w[s] = c[s]*exp(-gamma*s2[s]) on one row, then
            # replicate to 128 partitions via DRAM bounce.
            s2t_ps = ps.tile([NBLK, P], f32, tag="pm", name="s2t_ps")
            nc.tensor.transpose(s2t_ps[:], s2_pt[:], ident[:])
            s2_rows = row_p.tile([NBLK, P], f32, tag="s2rows", name="s2_rows")
            nc.vector.tensor_copy(out=s2_rows[:], in_=s2t_ps[:])
            s2_row = row_p.tile([1, SUPER], f32, tag="s2row", name="s2_row")
            nc.sync.dma_start(out=s2_row[:], in_=s2_rows[:])
            c_sl = row_p.tile([1, SUPER], f32, tag="csl", name="c_sl")
            nc.sync.dma_start(out=c_sl[:], in_=c_d[:, u * SUPER:(u + 1) * SUPER])
            w_exp = row_p.tile([1, SUPER], f32, tag="wexp", name="w_exp")
            nc.scalar.activation(out=w_exp[:], in_=s2_row[:], func=FT.Exp,
                                 scale=ng[:1, :])
            w_sl = row_p.tile([1, SUPER], bf16, tag="wsl", name="w_sl")
            nc.vector.tensor_mul(w_sl[:], w_exp[:], c_sl[:])
            nc.sync.dma_start(out=wrow_d[:, u * SUPER:(u + 1) * SUPER],
                              in_=w_sl[:])
            nc.sync.dma_start(
                out=w_bc[:, u * SUPER:(u + 1) * SUPER],
                in_=wrow_d[:, u * SUPER:(u + 1) * SUPER].partition_broadcast(P))

        # ---- main: matmuls + exp + weighted reduce ----
        for u in range(N_SUP):
            for t in range(NB):
                pm = ps.tile([P, EW], f32, tag="pm", name="pm")
                for fc in range(FC):
                    for h in range(EW // NT):
                        nc.tensor.matmul(
                            pm[:, h * NT:(h + 1) * NT],
                            xt[:, fc, t * P:(t + 1) * P],
                            st[:, fc, u * SUPER + h * NT:u * SUPER + (h + 1) * NT],
                            start=(fc == 0), stop=(fc == FC - 1))
                et = e_p.tile([P, EW], bf16, tag="et", name="et")
                nc.scalar.activation(out=et[:], in_=pm[:], func=FT.Exp,
                                     scale=two_g[:], bias=bias_pt[:, t:t + 1])
                dead = scr_p.tile([P, EW], bf16, tag="dead", name="dead")
                col = t * N_SUP + u
                nc.vector.scalar_tensor_tensor(
                    out=dead[:], in0=et[:], scalar=1.0,
                    in1=w_bc[:, u * SUPER:(u + 1) * SUPER],
                    op0=OP.mult, op1=OP.mult,
                    accum_out=parts[:, col:col + 1])

        # ---- finale: reduce partials, subtract rho, transpose out ----
        pv = parts[:].rearrange("p (t k) -> p t k", k=N_SUP)
        nc.vector.tensor_reduce(out=score[:], in_=pv,
                                axis=mybir.AxisListType.X, op=OP.add)
        nc.vector.tensor_scalar_sub(score[:], score[:], rb[:])
        sc_ps = ps.tile([NB, P], f32, tag="pm", name="sc_ps")
        nc.tensor.transpose(sc_ps[:], score[:], ident[:])
        sc_t = fin_p.tile([NB, P], f32, name="sc_t")
        nc.vector.tensor_copy(out=sc_t[:], in_=sc_ps[:])
        nc.sync.dma_start(out=out_d, in_=sc_t[:])

    nc.compile()
    _CACHE["nc"] = nc
    return nc


def _in_maps(inputs, support_vectors, coefficients, rho, gamma):
    x16 = np.asarray(inputs, dtype=np.float16)
    s16 = np.asarray(support_vectors, dtype=np.float16)
    c = np.ascontiguousarray(np.asarray(coefficients, np.float32)).reshape(1, S_TOT)
    r = np.asarray(rho, dtype=np.float32).reshape(1, 1)
    g = np.asarray(gamma, dtype=np.float32).reshape(1, 1)
    maps = []
    for cid in range(N_CORES):
        maps.append({
            "x": x16[cid * B_LOC:(cid + 1) * B_LOC],
            "s": s16[cid * S_SH:(cid + 1) * S_SH],
            "c": c,
            "rho": r,
            "gamma": g,
        })
    return maps


def _enable_jax_compile_cache():
    if _CACHE.get("jax_cc"):
        return
    try:
        import jax

        jax.config.update("jax_compilation_cache_dir", "/tmp/jax_bass_cc")
        jax.config.update("jax_persistent_cache_min_compile_time_secs", 0)
        jax.config.update("jax_persistent_cache_min_entry_size_bytes", -1)
    except Exception:
        pass
    _CACHE["jax_cc"] = True


def kernel(inputs, support_vectors, coefficients, rho, gamma, _trace=False):
    from concourse.bass_utils import run_bass_kernel_spmd

    _enable_jax_compile_cache()
    nc = _build()
    maps = _in_maps(inputs, support_vectors, coefficients, rho, gamma)
    res = run_bass_kernel_spmd(nc, maps, core_ids=list(range(N_CORES)),
                               trace=_trace)
    out = np.concatenate([np.asarray(r["out"], dtype=np.float32).reshape(B_LOC)
                          for r in res.results])
    if _trace:
        kernel.last_results = res
    return out
